# revision 39
# baseline (speedup 1.0000x reference)
"""Trainium2 Bass kernel for a dense transformer block (B=2, T=2048, E=1024, H=16).

Sharding: 8-way SPMD token split. Core c handles batch b=c//4 and the
stride-4 token residue class j=c%4 (tokens j::4, 512 per core) -- the
strided split balances causal-attention work exactly across the 4 cores of
a batch. K/V are computed only for the core's own 512 tokens and exchanged
within each 4-core batch group via fp8(e3m4) AllGather collectives (K, then
V in two halves), overlapped with the V/Q projections and the first pairs'
score/exp work so the exchange is largely off the critical path. Causality
with strided keys is handled by per-(key-rank) 128x128 triangular masks
supplied as per-core input data, so the SPMD program is identical on every
core.

Layout: activations feature-major [feat, tok]; all matmuls bf16 with fp32
PSUM accumulation; fp32 residual stream. LayerNorm stats via ones-column
matmuls on the PE (copy/square feeding them run on ScalarE); per-token
scale/offset rows broadcast to 128 partitions via K=1 bf16 matmuls.
Attention: Q/K in fp8e3 (the 1/sqrt(D) scale is folded into Exp's scale
operand so Q/K stay unit-variance); per head-pair, scoresT[key,query]
blocks land in a 2-bank PSUM tile so a single Exp instruction covers both
heads; AV accumulates token-major with a 65th ones-column in V giving the
softmax denominator for free; pair hp's score/exp/mask stream is software-
pipelined against pair hp-1's AV accumulations so ScalarE (exp, the
attention bottleneck) never starves. Weights are pre-transposed on the
host into the exact SBUF layout ([m, p, e, j]) so every weight DMA is a
contiguous slab copy; LN2 statistics are fused into the out-projection
loop.
"""

import numpy as np
import ml_dtypes
from contextlib import ExitStack

E, H, D = 1024, 16, 64
B, T = 2, 2048
EPS = 1e-5
TOK = 512            # tokens per core
NC_ = 8              # cores
GROUPS = [[0, 1, 2, 3], [4, 5, 6, 7]]
Bb = ml_dtypes.bfloat16

_compiled = {}


def _build_nc(with_bias):
    import concourse.bacc as bacc
    import concourse.mybir as mybir
    import concourse.tile as tile

    F32, BF16 = mybir.dt.float32, mybir.dt.bfloat16
    FP8 = mybir.dt.float8e3  # e3m4: max ~15.5, 4 mantissa bits
    AF = mybir.ActivationFunctionType

    nc = bacc.Bacc("TRN2", num_devices=NC_)

    xTq = nc.dram_tensor("xTq", [E, TOK], F32, kind="ExternalInput")
    wq_l = nc.dram_tensor("wq_l", [8, 128, 8, 128], BF16, kind="ExternalInput")
    wk_l = nc.dram_tensor("wk_l", [8, 128, 8, 128], BF16, kind="ExternalInput")
    wv_d = nc.dram_tensor("wv_d", [8, 128, E], BF16, kind="ExternalInput")
    wout_l = nc.dram_tensor("wout_l", [8, 128, 8, 128], BF16, kind="ExternalInput")
    wfc_l = nc.dram_tensor("wfc_l", [32, 128, 8, 128], BF16, kind="ExternalInput")
    wproj_l = nc.dram_tensor("wproj_l", [8, 128, 32, 128], BF16, kind="ExternalInput")
    masks_d = nc.dram_tensor("masks_d", [4, 128, 2, 128], BF16, kind="ExternalInput")
    if with_bias:
        bq_d = nc.dram_tensor("bq_d", [1, E], BF16, kind="ExternalInput")
        bk_d = nc.dram_tensor("bk_d", [1, E], BF16, kind="ExternalInput")
        bv_d = nc.dram_tensor("bv_d", [1, E], BF16, kind="ExternalInput")
        bout_d = nc.dram_tensor("bout_d", [1, E], BF16, kind="ExternalInput")
        bfc_d = nc.dram_tensor("bfc_d", [1, 4 * E], BF16, kind="ExternalInput")
        bproj_d = nc.dram_tensor("bproj_d", [1, E], BF16, kind="ExternalInput")
    yT = nc.dram_tensor("yT", [E, TOK], F32, kind="ExternalOutput")

    use_bias = bool(with_bias)

    with tile.TileContext(nc) as tc:
        with ExitStack() as octx:
            const = octx.enter_context(tc.tile_pool(name="const", bufs=1))
            ones_col_b = const.tile([128, 1], BF16)
            nc.vector.memset(ones_col_b[:], 1.0)
            ones_r128_b = const.tile([1, 128], BF16)
            nc.vector.memset(ones_r128_b[:], 1.0)
            ones64_b = const.tile([1, 64], BF16)
            nc.vector.memset(ones64_b[:], 1.0)
            if use_bias:
                ones_r512_b = const.tile([1, TOK], BF16)
                nc.vector.memset(ones_r512_b[:], 1.0)
            eps_t = const.tile([1, 1], F32)
            nc.vector.memset(eps_t[:], EPS)
            mk = [const.tile([128, 2, 128], BF16, tag=f"mk{jk}", name=f"mk{jk}")
                  for jk in range(4)]

            p_mid = octx.enter_context(tc.tile_pool(name="p_mid", bufs=1))
            xq = [p_mid.tile([128, TOK], F32, tag=f"xq{e}", name=f"xq{e}")
                  for e in range(8)]
            for e in range(8):
                nc.sync.dma_start(xq[e][:], xTq[e * 128:(e + 1) * 128, :])
            for jk in range(4):
                nc.sync.dma_start(mk[jk][:], masks_d[jk])
            attnT = [p_mid.tile([128, TOK], BF16, tag=f"attnT{e}", name=f"attnT{e}")
                     for e in range(8)]

            # flat fp8 bounce buffers: K and V exchanged in separate AGs so
            # the scores can start as soon as K lands while V is in flight
            KSZ = E * TOK                 # 1024*512 fp8 elems
            VSZ = TOK * 16 * 65
            dram = octx.enter_context(tc.tile_pool(name="dram", bufs=1, space="DRAM"))
            k_bi = dram.tile([KSZ], FP8)
            k_bo = dram.tile([4 * KSZ], FP8)
            VH = VSZ // 2
            v_bi = [dram.tile([VH], FP8, name=f"v_bi{h}") for h in range(2)]
            v_bo = [dram.tile([4 * VH], FP8, name=f"v_bo{h}") for h in range(2)]

            # FC weight tiles for the first half (created before the
            # attention-scoped pools so pool release stays LIFO). DMAs are
            # emitted early so they are done before the AG window opens.
            NPRE = 4
            p_wfc = octx.enter_context(tc.tile_pool(name="p_wfc", bufs=1))
            wfc_sb = [p_wfc.tile([128, 8, 128], BF16, tag=f"wfc{m}", name=f"wfc{m}")
                      for m in range(NPRE)]

            # ============ LayerNorm helper (feature-major) ============
            def ln_stats_apply(src_tiles, out_slabs, psum, wk, rows):
                """src_tiles[e]: f32 [128, TOK] sbuf; writes bf16 LN output."""
                st2 = psum.tile([65, TOK], F32, tag="lnps", name="st2", bufs=1)
                ps_sum, ps_sq = st2[0:1, :], st2[64:65, :]
                for e in range(8):
                    xb = wk.tile([128, TOK], BF16, tag=f"st_xb{e % 2}", name="st_xb")
                    nc.vector.tensor_copy(xb[:], src_tiles[e][:])
                    xq2 = wk.tile([128, TOK], BF16, tag=f"st_xq{e % 2}", name="st_xq")
                    nc.scalar.activation(xq2[:], src_tiles[e][:], AF.Square)
                    nc.tensor.matmul(ps_sum[:], ones_col_b[:], xb[:],
                                     start=(e == 0), stop=(e == 7))
                    nc.tensor.matmul(ps_sq[:], ones_col_b[:], xq2[:],
                                     start=(e == 0), stop=(e == 7))
                ln_rows_apply(st2, src_tiles, out_slabs, psum, wk, rows)

            def ln_rows_apply(st2, src_tiles, out_slabs, psum, wk, rows):
                ps_sum, ps_sq = st2[0:1, :], st2[64:65, :]
                mean = rows.tile([1, TOK], F32, tag="st_mean", name="st_mean")
                nc.scalar.activation(mean[:], ps_sum[:], AF.Copy, scale=1.0 / E)
                msq = rows.tile([1, TOK], F32, tag="st_msq", name="st_msq")
                nc.scalar.activation(msq[:], ps_sq[:], AF.Copy, scale=1.0 / E)
                m2 = rows.tile([1, TOK], F32, tag="st_m2", name="st_m2")
                nc.scalar.activation(m2[:], mean[:], AF.Square)
                var = rows.tile([1, TOK], F32, tag="st_var", name="st_var")
                nc.vector.tensor_sub(var[:], msq[:], m2[:])
                stdt = rows.tile([1, TOK], F32, tag="st_std", name="st_std")
                nc.scalar.activation(stdt[:], var[:], AF.Sqrt, bias=eps_t[:])
                r_row = rows.tile([1, TOK], F32, tag="st_r", name="st_r")
                nc.vector.reciprocal_approx_fast(r_row[:], stdt[:])
                o_row = rows.tile([1, TOK], F32, tag="st_o", name="st_o")
                nc.vector.tensor_mul(o_row[:], mean[:], r_row[:])
                rb_row = rows.tile([1, TOK], BF16, tag="st_rb", name="st_rb")
                nc.vector.tensor_copy(rb_row[:], r_row[:])
                ob_row = rows.tile([1, TOK], BF16, tag="st_ob", name="st_ob")
                nc.vector.tensor_copy(ob_row[:], o_row[:])
                ps_r = psum.tile([128, TOK], F32, tag="mm", name="ps_r", bufs=3)
                nc.tensor.matmul(ps_r[:], ones_r128_b[:], rb_row[:], start=True, stop=True)
                ps_o = psum.tile([128, TOK], F32, tag="mm", name="ps_o", bufs=3)
                nc.tensor.matmul(ps_o[:], ones_r128_b[:], ob_row[:], start=True, stop=True)
                r_bc = rows.tile([128, TOK], F32, tag="st_rbc", name="st_rbc")
                nc.scalar.activation(r_bc[:], ps_r[:], AF.Copy)
                o_bc = rows.tile([128, TOK], F32, tag="st_obc", name="st_obc")
                nc.scalar.activation(o_bc[:], ps_o[:], AF.Copy)
                for e in range(8):
                    tmp = wk.tile([128, TOK], F32, tag=f"st_tmp{e % 2}",
                                  name="st_tmp", bufs=2)
                    nc.vector.tensor_mul(tmp[:], src_tiles[e][:], r_bc[:])
                    nc.vector.tensor_sub(out_slabs[e][:], tmp[:], o_bc[:])

            # ==== Phase A: LN1, K proj, V proj -> combined AG, Q proj ====
            stackA = octx.enter_context(ExitStack())
            p_kv = stackA.enter_context(tc.tile_pool(name="p_kv", bufs=1))
            kT = [[p_kv.tile([128, TOK], FP8, tag=f"kT{jk}_{m}", name=f"kT{jk}_{m}")
                   for m in range(8)] for jk in range(4)]
            v_sb = [p_kv.tile([128, 16, 65], BF16, tag=f"v{t}", name=f"vt{t}")
                    for t in range(16)]
            stackV = stackA.enter_context(ExitStack())
            p_vf8 = stackV.enter_context(tc.tile_pool(name="p_vf8", bufs=6))
            qT = [p_kv.tile([128, TOK], FP8, tag=f"qT{m}", name=f"qT{m}")
                  for m in range(8)]

            with ExitStack() as ctx:
                wk = ctx.enter_context(tc.tile_pool(name="phA", bufs=1))
                rows = ctx.enter_context(tc.tile_pool(name="phAr", bufs=1))
                psum = ctx.enter_context(tc.tile_pool(name="phAps", bufs=1, space="PSUM"))
                bias_sb = ctx.enter_context(tc.tile_pool(name="phAb", bufs=1))
                if use_bias:
                    bq_sb = bias_sb.tile([1, E], BF16)
                    nc.sync.dma_start(bq_sb[:], bq_d[:])
                    bk_sb = bias_sb.tile([1, E], BF16)
                    nc.sync.dma_start(bk_sb[:], bk_d[:])
                    bv_sb = bias_sb.tile([1, E], BF16)
                    nc.sync.dma_start(bv_sb[:], bv_d[:])
                wv_sb = [bias_sb.tile([128, E], BF16, tag=f"wv{e}", name=f"wv{e}")
                         for e in range(8)]

                lnx = [bias_sb.tile([128, TOK], BF16, tag=f"lnx{e}", name=f"lnx{e}")
                       for e in range(8)]
                # prefetch half the q-proj weights to thin out AG-window DMA
                wqb = [bias_sb.tile([128, 8, 128], BF16, tag=f"wqb{m}",
                                    name=f"wqb{m}") for m in range(2)]
                for m in range(2):
                    nc.sync.dma_start(wqb[m][:], wq_l[m])
                ln_stats_apply(xq, lnx, psum, wk, rows)

                # --- K projection for own tokens (fp8 for the scores path) ---
                for m in range(8):
                    wblk = wk.tile([128, 8, 128], BF16, tag="wkblk", name="wkblk",
                                   bufs=3)
                    nc.sync.dma_start(wblk[:], wk_l[m])
                    ps = psum.tile([128, TOK], F32, tag="mm", name="kps", bufs=3)
                    if use_bias:
                        nc.tensor.matmul(ps[:], bk_sb[:, m * 128:(m + 1) * 128],
                                         ones_r512_b[:], start=True, stop=False)
                    for e in range(8):
                        nc.tensor.matmul(ps[:], wblk[:, e, :], lnx[e][:],
                                         start=(e == 0 and not use_bias),
                                         stop=(e == 7))
                    kloc = wk.tile([128, TOK], FP8, tag="kloc", name="kloc", bufs=3)
                    nc.vector.tensor_copy(kloc[:], ps[:])
                    nc.sync.dma_start(
                        k_bi[m * 128 * TOK:(m + 1) * 128 * TOK]
                        .rearrange("(p m) -> p m", p=128), kloc[:])
                nc.gpsimd.collective_compute(
                    "AllGather", mybir.AluOpType.bypass, replica_groups=GROUPS,
                    ins=[k_bi[:]], outs=[k_bo[:]])

                # --- V projection for own tokens ---
                for e in range(8):
                    nc.sync.dma_start(wv_sb[e][:], wv_d[e])
                for tl in range(4):
                    vloc = wk.tile([128, 16, 65], FP8, tag="vloc", name="vloc",
                                   bufs=2)
                    for nv in range(2):
                        ps = psum.tile([128, TOK], F32, tag="mm", name="vps", bufs=3)
                        if use_bias:
                            nc.tensor.matmul(
                                ps[:], ones_r128_b[:],
                                bv_sb[:, nv * 512:(nv + 1) * 512],
                                start=True, stop=False)
                        for e in range(8):
                            nc.tensor.matmul(
                                ps[:], lnx[e][:, tl * 128:(tl + 1) * 128],
                                wv_sb[e][:, nv * 512:(nv + 1) * 512],
                                start=(e == 0 and not use_bias), stop=(e == 7))
                        nc.vector.tensor_copy(
                            vloc[:, nv * 8:(nv + 1) * 8, 0:64],
                            ps[:].rearrange("p (h d) -> p h d", h=8))
                    nc.vector.memset(vloc[:, :, 64:65], 1.0)
                    nc.sync.dma_start(
                        v_bi[tl // 2][(tl % 2) * 128 * 1040:
                                      (tl % 2 + 1) * 128 * 1040]
                        .rearrange("(p m) -> p m", p=128),
                        vloc[:].rearrange("p h d -> p (h d)"))
                    if tl % 2 == 1:
                        nc.gpsimd.collective_compute(
                            "AllGather", mybir.AluOpType.bypass,
                            replica_groups=GROUPS,
                            ins=[v_bi[tl // 2][:]], outs=[v_bo[tl // 2][:]])
                for m in range(NPRE):
                    nc.sync.dma_start(wfc_sb[m][:], wfc_l[m])

                # --- Q projection for own tokens ---
                for m in range(8):
                    if m < 2:
                        wblk = wqb[m]
                    else:
                        wblk = wk.tile([128, 8, 128], BF16, tag="wkblk",
                                       name="wqblk", bufs=3)
                        nc.sync.dma_start(wblk[:], wq_l[m])
                    ps = psum.tile([128, TOK], F32, tag="mm", name="qps", bufs=3)
                    if use_bias:
                        nc.tensor.matmul(ps[:], bq_sb[:, m * 128:(m + 1) * 128],
                                         ones_r512_b[:], start=True, stop=False)
                    for e in range(8):
                        nc.tensor.matmul(ps[:], wblk[:, e, :], lnx[e][:],
                                         start=(e == 0 and not use_bias),
                                         stop=(e == 7))
                    nc.vector.tensor_copy(qT[m][:], ps[:])

                # --- fetch gathered K/V (uniform across cores) ---
                for jk in range(4):
                    for m in range(8):
                        off = jk * KSZ + m * 128 * TOK
                        nc.sync.dma_start(
                            kT[jk][m][:],
                            k_bo[off:off + 128 * TOK]
                            .rearrange("(p m) -> p m", p=128))
                # fetch v halves in p-major order so early AV blocks unblock
                for tl in range(4):
                    for jk in range(4):
                        tk = jk * 4 + tl
                        off = jk * VH + (tl % 2) * 128 * 1040
                        vf8 = p_vf8.tile([128, 16 * 65], FP8, tag="vf8", name="vf8")
                        nc.sync.dma_start(
                            vf8[:],
                            v_bo[tl // 2][off:off + 128 * 1040]
                            .rearrange("(p m) -> p m", p=128))
                        nc.vector.tensor_copy(
                            v_sb[tk][:].rearrange("p h d -> p (h d)"), vf8[:])
            stackV.close()  # vf8 staging dead once upcast to v_sb

            # ============ Attention (per head-pair, 16 key blocks) ============
            with ExitStack() as ctx:
                rows = ctx.enter_context(tc.tile_pool(name="atr", bufs=1))
                psum = ctx.enter_context(tc.tile_pool(name="atps", bufs=1, space="PSUM"))
                expool = ctx.enter_context(tc.tile_pool(name="expool", bufs=20))

                def norm_pair(hp, av):
                    dnb = rows.tile([1, 2, TOK], BF16, tag="dnb", name="dnb", bufs=2)
                    nc.vector.tensor_copy(dnb[:], av[64:65, :, :])
                    rb = psum.tile([64, 2, TOK], F32, tag="sc", name="rb", bufs=2)
                    for hh in range(2):
                        nc.tensor.matmul(rb[:, hh, :], ones64_b[:], dnb[:, hh, :],
                                         start=True, stop=True)
                    rbs = rows.tile([64, 2, TOK], F32, tag="rbs", name="rbs", bufs=2)
                    nc.vector.tensor_copy(rbs[:], rb[:])
                    rcp = rows.tile([64, 2, TOK], F32, tag="rcp", name="rcp", bufs=2)
                    nc.vector.reciprocal_approx_fast(rcp[:], rbs[:])
                    for hh in range(2):
                        nc.vector.tensor_mul(
                            attnT[hp][hh * 64:(hh + 1) * 64, :],
                            av[0:64, hh, :], rcp[:, hh, :])

                AV_ORDER = [(jk, p) for p in range(4) for jk in range(4)]

                def emit_av(hp, av, exs, j):
                    jk, p = AV_ORDER[j]
                    tk = jk * 4 + p
                    lo = 128 * p
                    nq = TOK - lo
                    for hh in range(2):
                        nc.tensor.matmul(av[:, hh, lo:lo + nq],
                                         v_sb[tk][:, hp * 2 + hh, :],
                                         exs[jk * 4 + p][:, hh, lo:lo + nq],
                                         start=(j == 0), stop=(j == 15))

                # software pipeline: pair hp's score/exp/mask blocks are
                # interleaved with pair hp-1's AV accumulations, so ScalarE
                # (exp) stays fed while the PE drains AVs, and the first
                # pair's scores overlap the V AllGather.
                prev = None
                for hp in range(8):
                    av = psum.tile([65, 2, TOK], F32, tag="av", name="av", bufs=2)
                    exs = []
                    for j, (jk, p) in enumerate(
                            (jk, p) for jk in range(4) for p in range(4)):
                        lo = 128 * p
                        nq = TOK - lo
                        sc = psum.tile([128, 2, TOK], F32, tag="sc", name="sc",
                                       bufs=2)
                        for hh in range(2):
                            po = hh * 64
                            nc.tensor.matmul(
                                sc[:, hh, lo:lo + nq],
                                kT[jk][hp][po:po + 64, lo:lo + 128],
                                qT[hp][po:po + 64, lo:lo + nq],
                                start=True, stop=True)
                        ex = expool.tile([128, 2, TOK], BF16, tag="ex", name="ex")
                        nc.scalar.activation(ex[:, :, lo:lo + nq],
                                             sc[:, :, lo:lo + nq], AF.Exp,
                                             scale=float(1.0 / np.sqrt(D)))
                        nc.vector.tensor_mul(ex[:, :, lo:lo + 128],
                                             ex[:, :, lo:lo + 128], mk[jk][:])
                        exs.append(ex)
                        if prev is not None:
                            emit_av(hp - 1, prev[0], prev[1], j)
                    if prev is not None:
                        norm_pair(hp - 1, prev[0])
                    prev = (av, exs)
                for j in range(16):
                    emit_av(7, prev[0], prev[1], j)
                norm_pair(7, prev[0])

            # ==== Out-proj + residual, LN2, FC+GELU, Proj + residual ====
            # single scope: no pool-teardown barriers between the phases
            stackA.close()  # free kT/v_sb/qT SBUF
            x1f = [p_mid.tile([128, TOK], F32, tag=f"x1f{e}", name=f"x1f{e}")
                   for e in range(8)]
            ln2x = [p_mid.tile([128, TOK], BF16, tag=f"ln2x{e}", name=f"ln2x{e}")
                    for e in range(8)]
            p_gel = octx.enter_context(tc.tile_pool(name="p_gel", bufs=1))
            gel = [p_gel.tile([128, TOK], BF16, tag=f"gel{k}", name=f"gel{k}")
                   for k in range(32)]
            with ExitStack() as ctx:
                wk = ctx.enter_context(tc.tile_pool(name="phM", bufs=3))
                rows = ctx.enter_context(tc.tile_pool(name="phMr", bufs=1))
                psum = ctx.enter_context(tc.tile_pool(name="phMps", bufs=1, space="PSUM"))
                bias_sb = ctx.enter_context(tc.tile_pool(name="phMb", bufs=1))
                if use_bias:
                    bout_sb = bias_sb.tile([1, E], BF16)
                    nc.sync.dma_start(bout_sb[:], bout_d[:])
                    bfc_sb = bias_sb.tile([1, 4 * E], BF16)
                    nc.sync.dma_start(bfc_sb[:], bfc_d[:])
                    bproj_sb = bias_sb.tile([1, E], BF16)
                    nc.sync.dma_start(bproj_sb[:], bproj_d[:])
                # out-proj with LN2 stats fused into the same loop
                st2 = psum.tile([65, TOK], F32, tag="lnps", name="st2b", bufs=1)
                for m in range(8):
                    wblk = wk.tile([128, 8, 128], BF16, tag="woblk", name="woblk")
                    nc.sync.dma_start(wblk[:], wout_l[m])
                    ps = psum.tile([128, TOK], F32, tag="mm", name="ops", bufs=3)
                    if use_bias:
                        nc.tensor.matmul(ps[:], bout_sb[:, m * 128:(m + 1) * 128],
                                         ones_r512_b[:], start=True, stop=False)
                    for e in range(8):
                        nc.tensor.matmul(ps[:], wblk[:, e, :], attnT[e][:],
                                         start=(e == 0 and not use_bias),
                                         stop=(e == 7))
                    nc.vector.tensor_add(x1f[m][:], ps[:], xq[m][:])
                    xb = wk.tile([128, TOK], BF16, tag=f"st_xb{m % 2}", name="st_xb")
                    nc.vector.tensor_copy(xb[:], x1f[m][:])
                    xq2 = wk.tile([128, TOK], BF16, tag=f"st_xq{m % 2}", name="st_xq")
                    nc.scalar.activation(xq2[:], x1f[m][:], AF.Square)
                    nc.tensor.matmul(st2[0:1, :], ones_col_b[:], xb[:],
                                     start=(m == 0), stop=(m == 7))
                    nc.tensor.matmul(st2[64:65, :], ones_col_b[:], xq2[:],
                                     start=(m == 0), stop=(m == 7))
                ln_rows_apply(st2, x1f, ln2x, psum, wk, rows)

                for m in range(32):
                    if m < NPRE:
                        wblk = wfc_sb[m]
                    else:
                        wblk = wk.tile([128, 8, 128], BF16, tag="wfblk",
                                       name="wfblk", bufs=3)
                        nc.sync.dma_start(wblk[:], wfc_l[m])
                    ps = psum.tile([128, TOK], F32, tag="mm", name="fps", bufs=3)
                    if use_bias:
                        nc.tensor.matmul(ps[:], bfc_sb[:, m * 128:(m + 1) * 128],
                                         ones_r512_b[:], start=True, stop=False)
                    for e in range(8):
                        nc.tensor.matmul(ps[:], wblk[:, e, :], ln2x[e][:],
                                         start=(e == 0 and not use_bias),
                                         stop=(e == 7))
                    nc.scalar.activation(gel[m][:], ps[:], AF.Gelu_apprx_tanh)

                for m in range(8):
                    wblk = wk.tile([128, 32, 128], BF16, tag="wpblk", name="wpblk",
                                   bufs=2)
                    nc.sync.dma_start(wblk[:], wproj_l[m])
                    ps = psum.tile([128, TOK], F32, tag="mm", name="pps", bufs=3)
                    if use_bias:
                        nc.tensor.matmul(ps[:], bproj_sb[:, m * 128:(m + 1) * 128],
                                         ones_r512_b[:], start=True, stop=False)
                    for k in range(32):
                        nc.tensor.matmul(ps[:], wblk[:, k, :], gel[k][:],
                                         start=(k == 0 and not use_bias),
                                         stop=(k == 31))
                    yt = wk.tile([128, TOK], F32, tag="yt", name="yt")
                    nc.vector.tensor_add(yt[:], ps[:], x1f[m][:])
                    nc.sync.dma_start(yT[m * 128:(m + 1) * 128, :], yt[:])

    nc.finalize()
    return nc


def _lhsT_blocks(w):
    """[K, M] weight -> [M//128, 128(p), K//128(e), 128(j)] bf16, so each
    m-block DMAs as a contiguous [128, K] slab in SBUF lhsT layout."""
    K, M = w.shape
    return np.ascontiguousarray(
        w.reshape(K // 128, 128, M // 128, 128).transpose(2, 1, 0, 3)).astype(Bb)


def _host_prep(x, ln1_g, ln1_b, w_qkv, b_qkv, w_out, b_out,
               ln2_g, ln2_b, w_fc, b_fc, w_proj, b_proj):
    f = np.float32
    x = np.asarray(x, f)
    w_qkv = np.asarray(w_qkv, f)
    b_qkv = np.asarray(b_qkv, f)
    w_out = np.asarray(w_out, f)
    b_out = np.asarray(b_out, f)
    w_fc = np.asarray(w_fc, f)
    b_fc = np.asarray(b_fc, f)
    w_proj = np.asarray(w_proj, f)
    b_proj = np.asarray(b_proj, f)

    # fold LN affine into the following matmul (exact):
    # (z*g + b) @ W = z @ (g[:,None]*W) + (b @ W)
    wqkv_eff = np.asarray(ln1_g, f)[:, None] * w_qkv
    bqkv_eff = np.asarray(ln1_b, f) @ w_qkv + b_qkv
    wfc_eff = np.asarray(ln2_g, f)[:, None] * w_fc
    bfc_eff = np.asarray(ln2_b, f) @ w_fc + b_fc

    cols = np.arange(3 * E).reshape(H, 3, D)
    qc, kc, vc = cols[:, 0, :].ravel(), cols[:, 1, :].ravel(), cols[:, 2, :].ravel()

    with_bias = any(
        np.abs(a).max() > 0
        for a in (bqkv_eff, b_out, bfc_eff, b_proj))
    shared = {
        "wq_l": _lhsT_blocks(wqkv_eff[:, qc]),
        "wk_l": _lhsT_blocks(wqkv_eff[:, kc]),
        "wv_d": wqkv_eff[:, vc].reshape(8, 128, E).astype(Bb),
        "wout_l": _lhsT_blocks(w_out),
        "wfc_l": _lhsT_blocks(wfc_eff),
        "wproj_l": _lhsT_blocks(w_proj),
    }
    if with_bias:
        shared.update({
            "bq_d": bqkv_eff[qc].reshape(1, -1).astype(Bb),
            "bk_d": bqkv_eff[kc].reshape(1, -1).astype(Bb),
            "bv_d": bqkv_eff[vc].reshape(1, -1).astype(Bb),
            "bout_d": b_out.reshape(1, -1).astype(Bb),
            "bfc_d": bfc_eff.reshape(1, -1).astype(Bb),
            "bproj_d": b_proj.reshape(1, -1).astype(Bb),
        })
    ml, ii = np.arange(128)[:, None], np.arange(128)[None, :]
    in_maps = []
    for c in range(NC_):
        b, jq = c // 4, c % 4
        masks = np.stack([
            np.broadcast_to(
                ((ii - (1 if jk > jq else 0)) >= ml)[:, None, :], (128, 2, 128))
            for jk in range(4)])
        m = {
            "xTq": np.ascontiguousarray(x[b, jq::4].T),
            "masks_d": np.ascontiguousarray(masks).astype(Bb),
        }
        m.update(shared)
        in_maps.append(m)
    return in_maps, with_bias


def kernel(**inputs):
    from concourse.bass_utils import run_bass_kernel_spmd

    in_maps, with_bias = _host_prep(**inputs)
    if ("nc", with_bias) not in _compiled:
        _compiled[("nc", with_bias)] = _build_nc(with_bias)
    nc = _compiled[("nc", with_bias)]
    res = run_bass_kernel_spmd(nc, in_maps, core_ids=list(range(NC_)))
    x = np.asarray(inputs["x"], np.float32)
    y = np.empty((B, T, E), np.float32)
    for c in range(NC_):
        b, jq = c // 4, c % 4
        y[b, jq::4, :] = res.results[c]["yT"].T
    return y
